# revision 1
# baseline (speedup 1.0000x reference)
"""ArDCA forward kernel for 8 trn2 NeuronCores.

z[m,i,a] = h[i,a] + sum_{j<i} sum_b J[i,j,b,a] * X[m,j,b]

Flattening (j,b)->K and (i,a)->N, this is one block-upper-triangular matmul
Z^T = Jmat^T @ X^T where J[i].reshape(L*Q, Q) is natively the i-th column
block of the stationary operand (no transpose of J needed).

Sharding: the 5376 output columns (i,a) are cut into 42 column-tiles of 128,
distributed over the 8 cores into 6 uniform slots per core (SPMD runs an
identical graph on every core; which column-tile a slot computes is decided
purely by the host-packed per-core J/h data — a slot whose tile needs fewer
K-tiles than the slot budget just gets zero-padded J). Each slot is one PSUM
accumulation chain: matmul(psum, lhsT=J_tile(128x128) bf16, rhs=XT_tile
(128x512) fp8) with f32 accumulation; a DVE tensor_scalar add of h evacuates
PSUM -> SBUF; the result is DMA'd out in bf16 (rel-err budget 2e-2, bf16
rounding adds ~2e-3) and upcast to f32 on the host. X^T (fp8: one-hot 0/1 is
exact) is resident in SBUF; J streams. All DRAM buffers are host-packed
partition-major so DMA descriptor runs per partition are >=512B.

Timing model (2.4 GHz runs): ~6.1us fixed NEFF preamble, ~5.8us warm-up
bridge covering the DMA ramp (first ~1MB needs that long: ~0.7us descriptor
gen per ring dma_start + ~250 GB/s early stream + completion latency), then
~28.5us of gapless matmuls at the 1 col/cycle PE roofline, ~3.7us tail
(last evac + store receipt + NEFF teardown). Budget padding (132 vs 113
k-tiles/core) is provably minimal under SPMD + shared-X addressing.
NOTE: the chip sometimes runs at 2.0 GHz (P-state); whole-run times then
scale by 1.2x. Compare runs via warm N=512 matmul duration (216 vs 259 ns).
"""

import math
import numpy as np
import ml_dtypes

M, L, Q = 512, 256, 21
LQ = L * Q                      # 5376 = 42*128
COLS = 128                      # output columns per group (column-tile)
NG = LQ // COLS                 # 42 column-tiles
NCORES = 8
NXT = LQ // 128                 # 42 X k-tiles
CKJ = 14                        # J k-tiles per DMA chunk
BF16 = ml_dtypes.bfloat16
FP8 = ml_dtypes.float8_e4m3


def _ktiles(g: int) -> int:
    i_max = (COLS * g + COLS - 1) // Q      # highest i in the tile
    return max(1, math.ceil(Q * i_max / 128))


def _plan():
    """Uniform slot structure + serpentine group->core assignment."""
    items = sorted(range(NG), key=lambda g: (-_ktiles(g), g))
    nslots = math.ceil(NG / NCORES)                      # 6
    budgets = [_ktiles(items[NCORES * r]) for r in range(nslots)]
    assign = [[None] * nslots for _ in range(NCORES)]    # assign[core][slot] = group
    for r in range(nslots):
        row = items[NCORES * r: NCORES * (r + 1)]
        for k, g in enumerate(row):
            core = k if r % 2 == 0 else NCORES - 1 - k
            assign[core][r] = g
    offs = [COLS * sum(budgets[:r]) for r in range(nslots)]  # jp col offset per slot
    return budgets, assign, offs


BUDGETS, ASSIGN, JOFFS = _plan()
S = len(BUDGETS)                 # 6 slots per core
WJ = COLS * sum(BUDGETS)         # jp total columns per core
WX = NXT * M                     # xt total columns (21504)
# ascending budgets: slot k first-touches only X tiles [B_{k-1}, B_k), so
# the X demand spreads over the whole run and the stream stays ahead of the
# PE with zero gaps; the final budget-42 chain also gives its first
# store-part ~5.9us of N=128 matmuls to hide under (a short final chain
# exposes both part-stores in the tail -- measured +1us)
SLOT_ORDER = sorted(range(S), key=lambda r: BUDGETS[r])
# X chunk k = the tiles slot k first-touches (tiles [maxB_so_far, B_k))
_cum = 0
XCHUNKS = []
for _r in SLOT_ORDER:
    XCHUNKS.append(max(0, BUDGETS[_r] - _cum))
    _cum = max(_cum, BUDGETS[_r])
XCHUNKS = tuple(XCHUNKS)
# jp columns laid out in consumption (SLOT_ORDER) order: the J stream is a
# single left-to-right sweep of jp
JOFFS = [0] * S
_off = 0
for _r in SLOT_ORDER:
    JOFFS[_r] = _off
    _off += BUDGETS[_r] * COLS


def _build_nc():
    import concourse.bacc as bacc
    import concourse.mybir as mybir
    from concourse import tile

    f32 = mybir.dt.float32
    bf16 = mybir.dt.bfloat16
    fp8 = mybir.dt.float8e4

    nc = bacc.Bacc(None, target_bir_lowering=False, debug=False)
    xt_ext = nc.declare_dram_parameter("xt", [128, WX], fp8, isOutput=False)
    jp_ext = nc.declare_dram_parameter("jp", [128, WJ], bf16, isOutput=False)
    hb_ext = nc.declare_dram_parameter("hb", [COLS, S], f32, isOutput=False)
    out_ext = nc.declare_dram_parameter("out", [S * COLS, M], bf16, isOutput=True)

    with tile.TileContext(nc) as tc:
        with (
            tc.tile_pool(name="x", bufs=1) as xpool,
            tc.tile_pool(name="j", bufs=1) as jpool,
            tc.tile_pool(name="ps", bufs=5, space="PSUM") as ppool,
            tc.tile_pool(name="psf", bufs=1, space="PSUM") as pfpool,
            tc.tile_pool(name="psw", bufs=1, space="PSUM") as pwpool,
            tc.tile_pool(name="o", bufs=6) as opool,
            tc.tile_pool(name="c", bufs=1) as cpool,
        ):

            # HAM warm-up bridge: the PE clock-gate releases (1.2 -> 2.4 GHz)
            # only after ~3.4us of sustained matmul activity, and the first
            # J/X pieces cannot arrive before ~2.5us after main-start (ring
            # descriptor-gen ~0.7us + transfer + completion).  Bridge exactly
            # that gap with dummy matmuls into a scratch PSUM bank; beyond it
            # they are a net loss -- real matmuls running through the cold
            # window cost less than burning full warm-ups (the cold PE also
            # consumes J/X at half rate, matching the ramping DMA supply).
            # The memset rides gpsimd, idle during the preamble, so the
            # bridge can start the moment the Tensor engine enters main.
            NWARM = 13
            zw = cpool.tile([128, 128], bf16, tag="zw")
            nc.vector.memset(zw[:], 0.0)
            # rhs for the dummies: the same 128 zero columns read 4x via a
            # zero-stride AP dim -> free size 512 with only a 32KB memset
            import concourse.bass as _bass
            _a = zw[:]
            zw_rhs = _bass.AP(_a.tensor, _a.offset,
                              [_a.ap[0], (0, M // 128), _a.ap[1]])
            hb_t = cpool.tile([COLS, S], f32, tag="hb")
            nc.gpsimd.dma_start(out=hb_t[:], in_=hb_ext[:])

            # one global DMA stream in exact consumption order, split over the
            # two HWDGE rings greedily by queued bytes (each ring is FIFO, so
            # balanced byte loads keep arrival order ~= consumption order);
            # every tile unique-tagged and resident (no pool-recycle waits)
            # one global DMA stream in exact consumption order, split over the
            # two HWDGE rings greedily by queued bytes (each ring is FIFO, so
            # balanced byte loads keep arrival order ~= consumption order);
            # small pieces on purpose: the early ones ramp up fastest, and
            # every piece signals its own completion so the PE never waits on
            # a mega-chunk. All tiles unique-tagged and resident.
            rings = [nc.sync, nc.scalar]
            ring_bytes = [0, 0]

            def ring_dma(out_ap, in_ap, nbytes):
                i = 0 if ring_bytes[0] <= ring_bytes[1] else 1
                rings[i].dma_start(out=out_ap, in_=in_ap)
                ring_bytes[i] += nbytes

            xts = []           # (tile, local_ktile) per global X ktile
            jtiles = {}        # (slot, chunk_start) -> (tile, col0)
            xoff = 0

            def emit_x(si, cx):
                nonlocal xoff
                # split a slot's fresh X window into <=4-tile items so arrival
                # is incremental. X stays on the HWDGE rings: SWDGE routing
                # was tried twice (full and hybrid) -- its early-pipeline
                # latency (~8us to first delivery: Q7 framework init +
                # serialized descriptor gen) starves even the slot-1 window,
                # and offloading only later windows does not shorten the
                # bridge (slot-2's first J chunk timing is unchanged)
                done = 0
                while done < cx:
                    n = min(4, cx - done)
                    xt_t = xpool.tile([128, n * M], fp8, tag=f"x{xoff}")
                    ring_dma(xt_t[:], xt_ext[:, xoff * M:(xoff + n) * M],
                             n * M * 128)
                    for t in range(n):
                        xts.append((xt_t, t))
                    xoff += n
                    done += n

            def jchunks(T):
                cs, t = [], 0
                while t < T:
                    ck = min(CKJ, T - t)
                    cs.append((t, ck))
                    t += ck
                return cs

            CHUNKS = {}

            def emit_j(r, t, ck):
                jt = jpool.tile([128, ck * COLS], bf16, tag=f"j{r}_{t}")
                c0 = JOFFS[r] + t * COLS
                ring_dma(jt[:], jp_ext[:, c0:c0 + ck * COLS], ck * COLS * 256)
                jtiles[(r, t)] = (jt, 0)

            for si, r in enumerate(SLOT_ORDER):
                T = BUDGETS[r]
                if si == 0:
                    emit_x(si, XCHUNKS[si])
                CHUNKS[r] = jchunks(T)
                for idx, (t, ck) in enumerate(CHUNKS[r]):
                    emit_j(r, t, ck)
                    if si > 0 and idx == 0 and XCHUNKS[si]:
                        emit_x(si, XCHUNKS[si])


            # warm-up bridge into a scratch bank (never read back)
            wps = pwpool.tile([COLS, M], f32, tag="wps")
            for w in range(NWARM):
                nc.tensor.matmul(wps[:], zw[:], zw_rhs,
                                 start=(w == 0), stop=(w == NWARM - 1))

            for si, r in enumerate(SLOT_ORDER):
                T = BUDGETS[r]
                if si == S - 1:
                    # final slot: split the chain by samples (N=384 then
                    # N=128, same J tiles). The wide chain's evac+store
                    # overlap the narrow chain's matmuls, so only the tiny
                    # N=128 evac+store is exposed after the last matmul.
                    # split point: part B's N=128 window (2.2us of matmuls)
                    # hides part A's evac + store-issue + HBM receipt
                    # (~2.3us, marginal but measured fine; HA=352 for extra
                    # margin showed no tail improvement -- the exposed tail
                    # is receipt/teardown-dominated either way)
                    HA = 384
                    ps_a = pfpool.tile([COLS, HA], f32, tag="psA")
                    ps_b = pfpool.tile([COLS, M - HA], f32, tag="psB")
                    ot = opool.tile([COLS, M], bf16, tag="ot")
                    rows = slice(r * COLS, (r + 1) * COLS)
                    for part, (ps_p, c0, c1, ring) in enumerate(
                        [(ps_a, 0, HA, nc.sync),
                         (ps_b, HA, M, nc.scalar)]
                    ):
                        for t, ck in CHUNKS[r]:
                            jt, jc0 = jtiles[(r, t)]
                            for tl in range(ck):
                                tt = t + tl
                                xt_t, xl = xts[tt]
                                nc.tensor.matmul(
                                    ps_p[:],
                                    jt[:, jc0 + tl * COLS:jc0 + (tl + 1) * COLS],
                                    xt_t[:, xl * M + c0:xl * M + c1],
                                    start=(tt == 0),
                                    stop=(tt == T - 1),
                                )
                        nc.vector.tensor_scalar_add(
                            ot[:, c0:c1], ps_p[:], hb_t[:, r:r + 1])
                        ring.dma_start(out=out_ext[rows, c0:c1],
                                       in_=ot[:, c0:c1])
                    continue
                ps = ppool.tile([COLS, M], f32, tag="ps")
                for t, ck in CHUNKS[r]:
                    jt, jc0 = jtiles[(r, t)]
                    for tl in range(ck):
                        tt = t + tl
                        xt_t, xl = xts[tt]
                        nc.tensor.matmul(
                            ps[:],
                            jt[:, jc0 + tl * COLS:jc0 + (tl + 1) * COLS],
                            xt_t[:, xl * M:(xl + 1) * M],
                            start=(tt == 0),
                            stop=(tt == T - 1),
                        )
                ot = opool.tile([COLS, M], bf16, tag="ot")
                # stores on SWDGE so they never delay the HWDGE load rings
                nc.vector.tensor_scalar_add(ot[:], ps[:], hb_t[:, r:r + 1])
                nc.gpsimd.dma_start(
                    out=out_ext[r * COLS:(r + 1) * COLS, :], in_=ot[:])

    nc.finalize()
    return nc


_CACHE = {}


def _get_nc():
    if "nc" not in _CACHE:
        _CACHE["nc"] = _build_nc()
    return _CACHE["nc"]


def _pack_inputs(X_oh, h_pos, J):
    """Build per-core in_maps (host-side shard + layout)."""
    XT = np.ascontiguousarray(X_oh.transpose(1, 2, 0).reshape(LQ, M))
    xt = np.ascontiguousarray(
        XT.reshape(NXT, 128, M).transpose(1, 0, 2).reshape(128, WX)
    ).astype(FP8)

    JT = J.reshape(L, LQ, Q).astype(BF16)   # JT[i] = (jb, a) column block of i
    h32 = h_pos.astype(np.float32)

    in_maps = []
    for core in range(NCORES):
        jp = np.zeros((128, WJ), dtype=BF16)
        hb = np.zeros((COLS, S), dtype=np.float32)
        for r in range(S):
            g = ASSIGN[core][r]
            if g is None:
                continue
            T = BUDGETS[r]
            blk = np.zeros((T * 128, COLS), dtype=BF16)
            # columns are global output indices ia = COLS*g + col, i = ia//Q
            ia0 = COLS * g
            col = 0
            while col < COLS:
                i, a0 = divmod(ia0 + col, Q)
                na = min(Q - a0, COLS - col)        # run of columns within one i
                rows = Q * i                        # strictly-lower mask: j < i
                blk[:rows, col:col + na] = JT[i][:rows, a0:a0 + na]
                hb[col:col + na, r] = h32[i, a0:a0 + na]
                col += na
            jp[:, JOFFS[r]:JOFFS[r] + T * COLS] = (
                blk.reshape(T, 128, COLS).transpose(1, 0, 2).reshape(128, T * COLS)
            )
        in_maps.append({"xt": xt, "jp": jp, "hb": hb})
    return in_maps


def _unpack_outputs(results):
    outT = np.zeros((LQ, M), dtype=np.float32)
    for core in range(NCORES):
        o = np.asarray(results[core]["out"]).astype(np.float32)
        for r in range(S):
            g = ASSIGN[core][r]
            if g is None:
                continue
            outT[COLS * g:COLS * (g + 1)] = o[r * COLS:(r + 1) * COLS]
    return np.ascontiguousarray(outT.reshape(L, Q, M).transpose(2, 0, 1))


def _run(in_maps, trace=False, **kw):
    from concourse.bass_utils import run_bass_kernel_spmd

    nc = _get_nc()
    return run_bass_kernel_spmd(nc, in_maps, list(range(NCORES)), trace=trace, **kw)


def kernel(X_oh, h_pos, J):
    X_oh = np.asarray(X_oh, dtype=np.float32)
    h_pos = np.asarray(h_pos, dtype=np.float32)
    J = np.asarray(J, dtype=np.float32)
    in_maps = _pack_inputs(X_oh, h_pos, J)
    res = _run(in_maps)
    return _unpack_outputs(res.results)



# revision 2
# speedup vs baseline: 1.1288x; 1.1288x over previous
"""ArDCA forward kernel for 8 trn2 NeuronCores.

z[m,i,a] = h[i,a] + sum_{j<i} sum_b J[i,j,b,a] * X[m,j,b]

Flattening (j,b)->K and (i,a)->N, this is one block-upper-triangular matmul
Z^T = Jmat^T @ X^T where J[i].reshape(L*Q, Q) is natively the i-th column
block of the stationary operand (no transpose of J needed).

Sharding: the 5376 output columns (i,a) are cut into 42 column-tiles of 128,
distributed over the 8 cores into 6 uniform slots per core (SPMD runs an
identical graph on every core; which column-tile a slot computes is decided
purely by the host-packed per-core J/h data — a slot whose tile needs fewer
K-tiles than the slot budget just gets zero-padded J). Each slot is one PSUM
accumulation chain; a DVE tensor_scalar add of h evacuates PSUM -> SBUF; the
result is DMA'd out in bf16 and upcast to f32 on the host.

Mixed precision (rel-err budget 2e-2): X^T is packed fp8 with hot value
2^-6 (exact in e4m3) and J is packed scaled by 64 (power of two: exact),
so bf16 and fp8 products are both at true scale and share one PSUM chain.
K-tiles < TH=18 of each chain stay bf16 (regular matmul, 1 k-tile / 512
cycles); k-tiles >= TH are fp8e4m3 and run as DoubleRow pairs (2 k-tiles /
512 cycles, fp8 double-pump). Host-measured exact rel err of this split on
the fixed problem seed: 1.51e-2 (bf16-only: 1.7e-3; fp8-all: 2.64e-2).

Timing model (2.4 GHz): ~7us fixed NEFF preamble, ~4us warm-up bridge
covering the DMA ramp + HAM clock ungating, then gapless matmuls at the
1 col/cycle PE roofline (84 regular + 24 DoubleRow slot-times = 108 x 216ns
~ 23.3us), ~5us tail (last evac + store receipt + NEFF teardown).
"""

import math
import numpy as np
import ml_dtypes

M, L, Q = 512, 256, 21
LQ = L * Q                      # 5376 = 42*128
COLS = 128                      # output columns per group (column-tile)
NG = LQ // COLS                 # 42 column-tiles
NCORES = 8
NXT = LQ // 128                 # 42 X k-tiles
CKJ = 14                        # J k-tiles per DMA chunk
TH = 18                         # k-tiles below TH stay bf16; >= TH are fp8 pairs
XHOT = 0.015625                 # 2^-6: one-hot value, exact in fp8e4m3
JSCL = 64.0                     # J pre-scale (power of 2; cancels XHOT)
BF16 = ml_dtypes.bfloat16
FP8 = ml_dtypes.float8_e4m3


def _ktiles(g: int) -> int:
    i_max = (COLS * g + COLS - 1) // Q      # highest i in the tile
    return max(1, math.ceil(Q * i_max / 128))


def _plan():
    """Uniform slot structure + serpentine group->core assignment."""
    items = sorted(range(NG), key=lambda g: (-_ktiles(g), g))
    nslots = math.ceil(NG / NCORES)                      # 6
    budgets = [_ktiles(items[NCORES * r]) for r in range(nslots)]
    assign = [[None] * nslots for _ in range(NCORES)]    # assign[core][slot] = group
    for r in range(nslots):
        row = items[NCORES * r: NCORES * (r + 1)]
        for k, g in enumerate(row):
            core = k if r % 2 == 0 else NCORES - 1 - k
            assign[core][r] = g
    return budgets, assign


BUDGETS, ASSIGN = _plan()
S = len(BUDGETS)                 # 6 slots per core
WX = NXT * M                     # xt total columns (21504)
# ascending budgets: slot k first-touches only X tiles [B_{k-1}, B_k), so
# the X demand spreads over the whole run and the stream stays ahead of the
# PE with zero gaps
SLOT_ORDER = sorted(range(S), key=lambda r: BUDGETS[r])
_cum = 0
XCHUNKS = []
for _r in SLOT_ORDER:
    XCHUNKS.append(max(0, BUDGETS[_r] - _cum))
    _cum = max(_cum, BUDGETS[_r])
XCHUNKS = tuple(XCHUNKS)
# per-slot bf16 / fp8 k-tile counts and packed column offsets, laid out in
# consumption (SLOT_ORDER) order so each J stream is a left-to-right sweep
N16 = [min(BUDGETS[r], TH) for r in range(S)]
N8 = [max(0, BUDGETS[r] - TH) for r in range(S)]
assert all(n % 2 == 0 for n in N8 if n)
J16OFFS = [0] * S
J8OFFS = [0] * S
_o16 = _o8 = 0
for _r in SLOT_ORDER:
    J16OFFS[_r] = _o16
    J8OFFS[_r] = _o8
    _o16 += N16[_r] * COLS
    _o8 += N8[_r] * COLS
W16, W8 = _o16, _o8


def _build_nc():
    import concourse.bacc as bacc
    import concourse.mybir as mybir
    from concourse import tile

    f32 = mybir.dt.float32
    bf16 = mybir.dt.bfloat16
    fp8 = mybir.dt.float8e4
    DR = mybir.MatmulPerfMode.DoubleRow

    nc = bacc.Bacc(None, target_bir_lowering=False, debug=False)
    xt_ext = nc.declare_dram_parameter("xt", [128, WX], fp8, isOutput=False)
    j16_ext = nc.declare_dram_parameter("j16", [128, W16], bf16, isOutput=False)
    j8_ext = nc.declare_dram_parameter("j8", [128, W8], fp8, isOutput=False)
    hb_ext = nc.declare_dram_parameter("hb", [COLS, S], f32, isOutput=False)
    out_ext = nc.declare_dram_parameter("out", [S * COLS, M], bf16, isOutput=True)

    with tile.TileContext(nc) as tc:
        with (
            tc.tile_pool(name="x", bufs=1) as xpool,
            tc.tile_pool(name="j", bufs=1) as jpool,
            tc.tile_pool(name="ps", bufs=5, space="PSUM") as ppool,
            tc.tile_pool(name="psf", bufs=1, space="PSUM") as pfpool,
            tc.tile_pool(name="psw", bufs=1, space="PSUM") as pwpool,
            tc.tile_pool(name="o", bufs=6) as opool,
            tc.tile_pool(name="c", bufs=1) as cpool,
        ):

            # HAM warm-up bridge: the PE clock-gate releases (1.2 -> 2.4 GHz)
            # only after ~3.4us of sustained matmul activity, and the first
            # J/X pieces cannot arrive before ~2.5us after main-start (ring
            # descriptor-gen ~0.7us + transfer + completion).  Bridge exactly
            # that gap with dummy matmuls into a scratch PSUM bank.
            NWARM = 13
            zw = cpool.tile([128, 128], bf16, tag="zw")
            nc.vector.memset(zw[:], 0.0)
            # rhs for the dummies: the same 128 zero columns read 4x via a
            # zero-stride AP dim -> free size 512 with only a 32KB memset
            import concourse.bass as _bass
            _a = zw[:]
            zw_rhs = _bass.AP(_a.tensor, _a.offset,
                              [_a.ap[0], (0, M // 128), _a.ap[1]])
            hb_t = cpool.tile([COLS, S], f32, tag="hb")
            nc.gpsimd.dma_start(out=hb_t[:], in_=hb_ext[:])

            # one global DMA stream in exact consumption order, split over the
            # two HWDGE rings greedily by queued bytes (each ring is FIFO, so
            # balanced byte loads keep arrival order ~= consumption order);
            # small pieces on purpose: the early ones ramp up fastest, and
            # every piece signals its own completion so the PE never waits on
            # a mega-chunk. All tiles unique-tagged and resident.
            rings = [nc.sync, nc.scalar]
            ring_bytes = [0, 0]

            def ring_dma(out_ap, in_ap, nbytes):
                i = 0 if ring_bytes[0] <= ring_bytes[1] else 1
                rings[i].dma_start(out=out_ap, in_=in_ap)
                ring_bytes[i] += nbytes

            xts = []            # (tile, local_ktile) per global X ktile
            j16tiles = {}       # (slot, chunk_start) -> tile
            j8tiles = {}
            xoff = 0

            def emit_x(si, cx):
                nonlocal xoff
                # split a slot's fresh X window into <=4-tile items so arrival
                # is incremental. X stays on the HWDGE rings (SWDGE's early
                # pipeline latency starves the first windows).
                done = 0
                while done < cx:
                    n = min(4, cx - done)
                    xt_t = xpool.tile([128, n, M], fp8, tag=f"x{xoff}")
                    ring_dma(xt_t[:], xt_ext[:, xoff * M:(xoff + n) * M],
                             n * M * 128)
                    for t in range(n):
                        xts.append((xt_t, t))
                    xoff += n
                    done += n

            def jchunks(T):
                cs, t = [], 0
                while t < T:
                    ck = min(CKJ, T - t)
                    cs.append((t, ck))
                    t += ck
                return cs

            CHUNKS16 = {}
            CHUNKS8 = {}

            def emit_j16(r, t, ck):
                jt = jpool.tile([128, ck, COLS], bf16, tag=f"j{r}_{t}")
                c0 = J16OFFS[r] + t * COLS
                ring_dma(jt[:], j16_ext[:, c0:c0 + ck * COLS], ck * COLS * 256)
                j16tiles[(r, t)] = jt

            def emit_j8(r, t, ck):
                # t is the chunk offset within the fp8 region (global k-tile
                # TH + t); ck is even (pairs never straddle chunks)
                jt = jpool.tile([128, ck, COLS], fp8, tag=f"j8{r}_{t}")
                c0 = J8OFFS[r] + t * COLS
                ring_dma(jt[:], j8_ext[:, c0:c0 + ck * COLS], ck * COLS * 128)
                j8tiles[(r, t)] = jt

            for si, r in enumerate(SLOT_ORDER):
                if si == 0:
                    emit_x(si, XCHUNKS[si])
                CHUNKS16[r] = jchunks(N16[r])
                CHUNKS8[r] = jchunks(N8[r])
                for idx, (t, ck) in enumerate(CHUNKS16[r]):
                    emit_j16(r, t, ck)
                    if si > 0 and idx == 0 and XCHUNKS[si]:
                        emit_x(si, XCHUNKS[si])
                for t, ck in CHUNKS8[r]:
                    emit_j8(r, t, ck)

            # warm-up bridge into a scratch bank (never read back)
            wps = pwpool.tile([COLS, M], f32, tag="wps")
            for w in range(NWARM):
                nc.tensor.matmul(wps[:], zw[:], zw_rhs,
                                 start=(w == 0), stop=(w == NWARM - 1))

            def emit_slot_matmuls(r, ps_p, c0, c1):
                """All matmuls of slot r restricted to sample cols [c0, c1)."""
                T = BUDGETS[r]
                for t, ck in CHUNKS16[r]:
                    jt = j16tiles[(r, t)]
                    for tl in range(ck):
                        tt = t + tl
                        xt_t, xl = xts[tt]
                        nc.tensor.matmul(
                            ps_p[:],
                            jt[:, tl, :],
                            xt_t[:, xl, c0:c1],
                            start=(tt == 0),
                            stop=(tt == T - 1),
                        )
                for t, ck in CHUNKS8[r]:
                    jt = j8tiles[(r, t)]
                    for tl in range(0, ck, 2):
                        tt = TH + t + tl            # global k-tile of the pair
                        xt_t, xl = xts[tt]
                        xt_t2, xl2 = xts[tt + 1]
                        assert xt_t2 is xt_t and xl2 == xl + 1
                        nc.tensor.matmul(
                            ps_p[:],
                            jt[:, tl:tl + 2, :],
                            xt_t[:, xl:xl + 2, c0:c1],
                            start=False,
                            stop=(tt + 2 == T),
                            perf_mode=DR,
                        )

            for si, r in enumerate(SLOT_ORDER):
                if si == S - 1:
                    # final slot: split the chain by samples (N=384 then
                    # N=128, same J tiles). The wide chain's evac+store
                    # overlap the narrow chain's matmuls, so only the tiny
                    # N=128 evac+store is exposed after the last matmul.
                    HA = 384
                    ps_a = pfpool.tile([COLS, HA], f32, tag="psA")
                    ps_b = pfpool.tile([COLS, M - HA], f32, tag="psB")
                    ot = opool.tile([COLS, M], bf16, tag="ot")
                    rows = slice(r * COLS, (r + 1) * COLS)
                    for part, (ps_p, c0, c1, ring) in enumerate(
                        [(ps_a, 0, HA, nc.sync),
                         (ps_b, HA, M, nc.scalar)]
                    ):
                        emit_slot_matmuls(r, ps_p, c0, c1)
                        nc.vector.tensor_scalar_add(
                            ot[:, c0:c1], ps_p[:], hb_t[:, r:r + 1])
                        ring.dma_start(out=out_ext[rows, c0:c1],
                                       in_=ot[:, c0:c1])
                    continue
                ps = ppool.tile([COLS, M], f32, tag="ps")
                emit_slot_matmuls(r, ps, 0, M)
                ot = opool.tile([COLS, M], bf16, tag="ot")
                # stores on SWDGE so they never delay the HWDGE load rings
                nc.vector.tensor_scalar_add(ot[:], ps[:], hb_t[:, r:r + 1])
                nc.gpsimd.dma_start(
                    out=out_ext[r * COLS:(r + 1) * COLS, :], in_=ot[:])

    nc.finalize()
    return nc


_CACHE = {}


def _get_nc():
    if "nc" not in _CACHE:
        _CACHE["nc"] = _build_nc()
    return _CACHE["nc"]


def _pack_inputs(X_oh, h_pos, J):
    """Build per-core in_maps (host-side shard + layout)."""
    XT = np.ascontiguousarray(X_oh.transpose(1, 2, 0).reshape(LQ, M)) * XHOT
    xt = np.ascontiguousarray(
        XT.reshape(NXT, 128, M).transpose(1, 0, 2).reshape(128, WX)
    ).astype(FP8)

    Js = (J * JSCL).astype(np.float32)
    JT = Js.reshape(L, LQ, Q)   # JT[i] = (jb, a) column block of i
    h32 = h_pos.astype(np.float32)

    in_maps = []
    for core in range(NCORES):
        j16 = np.zeros((128, W16), dtype=BF16)
        j8 = np.zeros((128, W8), dtype=FP8)
        hb = np.zeros((COLS, S), dtype=np.float32)
        for r in range(S):
            g = ASSIGN[core][r]
            if g is None:
                continue
            T = BUDGETS[r]
            blk = np.zeros((T * 128, COLS), dtype=np.float32)
            # columns are global output indices ia = COLS*g + col, i = ia//Q
            ia0 = COLS * g
            col = 0
            while col < COLS:
                i, a0 = divmod(ia0 + col, Q)
                na = min(Q - a0, COLS - col)        # run of columns within one i
                rows = Q * i                        # strictly-lower mask: j < i
                blk[:rows, col:col + na] = JT[i][:rows, a0:a0 + na]
                hb[col:col + na, r] = h32[i, a0:a0 + na]
                col += na
            b3 = blk.reshape(T, 128, COLS)
            n16, n8 = N16[r], N8[r]
            j16[:, J16OFFS[r]:J16OFFS[r] + n16 * COLS] = (
                b3[:n16].transpose(1, 0, 2).reshape(128, n16 * COLS)
            ).astype(BF16)
            if n8:
                j8[:, J8OFFS[r]:J8OFFS[r] + n8 * COLS] = (
                    b3[TH:].transpose(1, 0, 2).reshape(128, n8 * COLS)
                ).astype(FP8)
        in_maps.append({"xt": xt, "j16": j16, "j8": j8, "hb": hb})
    return in_maps


def _unpack_outputs(results):
    outT = np.zeros((LQ, M), dtype=np.float32)
    for core in range(NCORES):
        o = np.asarray(results[core]["out"]).astype(np.float32)
        for r in range(S):
            g = ASSIGN[core][r]
            if g is None:
                continue
            outT[COLS * g:COLS * (g + 1)] = o[r * COLS:(r + 1) * COLS]
    return np.ascontiguousarray(outT.reshape(L, Q, M).transpose(2, 0, 1))


def _run(in_maps, trace=False, **kw):
    from concourse.bass_utils import run_bass_kernel_spmd

    nc = _get_nc()
    return run_bass_kernel_spmd(nc, in_maps, list(range(NCORES)), trace=trace, **kw)


def kernel(X_oh, h_pos, J):
    X_oh = np.asarray(X_oh, dtype=np.float32)
    h_pos = np.asarray(h_pos, dtype=np.float32)
    J = np.asarray(J, dtype=np.float32)
    in_maps = _pack_inputs(X_oh, h_pos, J)
    res = _run(in_maps)
    return _unpack_outputs(res.results)
